# revision 23
# baseline (speedup 1.0000x reference)
"""MoE expert-FFN kernel for Trainium2, expert-parallel across 8 NeuronCores.

Problem: out[t] = silu(x[t] @ W1[e_t]^T) @ W2[e_t]^T with
  E=64 experts, D=512, H=1024, T=256 tokens.

Strategy (memory-bound on expert weights):
  - Core c owns experts [8c, 8c+8). Host routes tokens to the core owning
    their expert, padding each expert's tokens to a fixed capacity C.
  - Host pre-packs weights into the exact SBUF layout in FP16 (halves the
    HBM traffic vs fp32; PSUM accumulation stays fp32, absmax rel err
    ~5e-4 vs the fp32 oracle, well inside the 2e-2 budget).
  - Experts are processed in groups of 4 so every on-chip tile is a full
    128-partition tile (4 experts x 32-token capacity):
      fc1: weights are the MOVING matmul operand streamed against the
           stationary token block; the 4 experts of a group stream
           concurrently through distinct 32-column PE groups
           (tile_position=(0,32s)).
      silu: one ACT op per group, PSUM [128,1024] -> SBUF fp16.
      transpose: one [128,128] PE transpose per h-chunk (8 per group).
      fc2: 8 independent per-expert accumulation chains.
  - DMA schedule is shaped so the last-arriving bytes unblock the least
    remaining work: both groups' W1 go first (2 x 4MiB), then W2
    per-expert in 512KiB halves (256KiB quarters for the final expert)
    on the sync HWDGE ring; x/identity ride the scalar ring so the
    weight stream starts immediately; each expert's fc2 chain fires as
    its own W2 slices land, so after the final byte only a couple of
    matmuls + a split cast + a 32KiB store remain. The per-expert fc2
    PSUM tiles alternate banks so the PE never stalls behind the
    previous expert's PSUM->SBUF cast (same-bank PE-write/DVE-read
    hazard would otherwise serialize chain k+1 behind cast k).

Measured on 8 axon-tunneled trn2 cores: ~60-65us (vs 251us fp32
baseline), absmax rel err ~4.8e-4 (budget 2e-2). The weight stream runs
gapless at 305-400GB/s/core (HBM-arbitration dependent); ~8.5us NEFF
preamble and ~4us teardown are framework-fixed.
"""

import numpy as np

E, D, H, T = 64, 512, 1024, 256
NCORES = 8
EPC = E // NCORES          # experts per core (8)
GPE = 4                    # experts per group
NG = EPC // GPE            # groups per core (2)
DC = D // 128              # 4 d-chunks
HC = H // 128              # 8 h-chunks
WE = DC * H                # 4096 cols of packed W1 (= HC*D for W2) per expert
CB = 32                    # token block (col-tile granularity)

_prog_cache = {}


def _build_program(C, dual=False):
    # dual=True (weight stream split across both HWDGE rings) was A/B
    # tested and measured ~13us slower; single-ring is the keeper.
    import concourse.mybir as mybir
    import concourse.tile as tile
    from concourse import bacc

    f32 = mybir.dt.float32
    f16 = mybir.dt.float16
    blocks = C // CB
    nc = bacc.Bacc("TRN2", target_bir_lowering=False, debug=False)

    def wdma(tile_, dram, c0, c1):
        # Weight DMA of tile cols [c0:c1]: optionally split by columns
        # across both HWDGE rings (sync + scalar) to raise this core's
        # HBM arbitration share under contention.
        if not dual:
            nc.sync.dma_start(tile_[:, c0:c1], dram[:, c0:c1])
            return
        m = (c0 + c1) // 2
        nc.sync.dma_start(tile_[:, c0:m], dram[:, c0:m])
        nc.scalar.dma_start(tile_[:, m:c1], dram[:, m:c1])

    w1t_d = nc.dram_tensor("w1t", [NG, 128, GPE * WE], f16, kind="ExternalInput")
    w2t_d = nc.dram_tensor("w2t", [EPC, 128, WE], f16, kind="ExternalInput")
    xt = nc.dram_tensor("xt", [128, EPC * DC * C], f16, kind="ExternalInput")
    idt = nc.dram_tensor("idt", [128, 128], f16, kind="ExternalInput")
    yt = nc.dram_tensor("yt", [EPC, blocks, CB, D], f16, kind="ExternalOutput")

    with tile.TileContext(nc) as tc:
        with (
            tc.tile_pool(name="w1pool", bufs=NG) as w1pool,
            tc.tile_pool(name="w2pool", bufs=EPC) as w2pool,
            tc.tile_pool(name="xpool", bufs=2) as xpool,
            tc.tile_pool(name="hpool", bufs=2) as hpool,
            tc.tile_pool(name="tpool", bufs=2 * blocks) as tpool,
            tc.tile_pool(name="ypool", bufs=4) as ypool,
            tc.tile_pool(name="psh", bufs=2, space="PSUM") as pshp,
            tc.tile_pool(name="pst", bufs=2, space="PSUM") as pstp,
            tc.tile_pool(name="psy", bufs=2, space="PSUM") as psyp,
        ):
            # x + identity ride the scalar HWDGE ring; the weight stream
            # owns the sync ring from t=0.
            ident = xpool.tile([128, 128], f16)
            nc.scalar.dma_start(ident[:], idt[:])
            ident_w = ident[:]
            xall = xpool.tile([128, EPC * DC * C], f16)
            nc.scalar.dma_start(xall[:], xt[:])

            w1 = []
            for g in range(NG):
                w = w1pool.tile([128, GPE * WE], f16, tag="w1")
                wdma(w, w1t_d[g], 0, GPE * WE)
                w1.append(w)
            # W2 arrives in 512KiB halves (quarters for the final expert)
            # so the tail chain only waits on a sliver of weights when the
            # last bytes land.
            w2 = []
            for e in range(EPC):
                w = w2pool.tile([128, WE], f16, tag="w2")
                nparts = 4 if e >= EPC - GPE else 2
                step = WE // nparts
                for p in range(nparts):
                    wdma(w, w2t_d[e], p * step, (p + 1) * step)
                w2.append(w)

            # ---- fc1 + silu + transpose for both groups
            hts = {}
            for g in range(NG):
                for b in range(blocks):
                    psh = pshp.tile([128, H], f32, tag="psh")
                    for c in range(DC):
                        for s in range(GPE):
                            sg = g * GPE + s
                            xsl = xall[:, (sg * DC + c) * C + b * CB:
                                       (sg * DC + c) * C + (b + 1) * CB]
                            for nh in range(2):
                                nc.tensor.matmul(
                                    psh[32 * s:32 * (s + 1),
                                        nh * 512:(nh + 1) * 512],
                                    xsl,
                                    w1[g][:, s * WE + c * H + nh * 512:
                                          s * WE + c * H + (nh + 1) * 512],
                                    start=(c == 0),
                                    stop=(c == DC - 1),
                                    tile_position=(0, 32 * s),
                                )

                    hbuf = hpool.tile([128, H], f16, tag="h")
                    nc.scalar.activation(
                        hbuf[:], psh[:], mybir.ActivationFunctionType.Silu
                    )

                    pst = pstp.tile([128, H], f16, tag="pst")
                    for ch in range(HC):
                        nc.tensor.transpose(
                            pst[:, ch * 128:(ch + 1) * 128],
                            hbuf[:, ch * 128:(ch + 1) * 128],
                            ident_w,
                        )
                    ht = tpool.tile([128, H], f16, tag="ht")
                    nc.vector.tensor_copy(ht[:], pst[:])
                    hts[(g, b)] = ht

            # ---- fc2: per-expert accumulation chains, in weight-arrival
            # order, so each chain fires as its own 1MiB W2 lands. Each
            # chain gets its own PSUM tile (banks alternate) so the PE
            # never stalls behind the previous expert's PSUM->SBUF cast.
            for g in range(NG):
                for b in range(blocks):
                    ht = hts[(g, b)]
                    for s in range(GPE):
                        e = g * GPE + s
                        psy = psyp.tile([CB, D], f32, tag="psy")
                        for ch in range(HC):
                            nc.tensor.matmul(
                                psy[:],
                                ht[:, ch * 128 + 32 * s:
                                   ch * 128 + 32 * (s + 1)],
                                w2[e][:, ch * D:(ch + 1) * D],
                                start=(ch == 0),
                                stop=(ch == HC - 1),
                            )
                        ybuf = ypool.tile([CB, D], f16, tag="y")
                        final = (g == NG - 1 and s == GPE - 1
                                 and b == blocks - 1)
                        if final:
                            # tail expert: split the PSUM->SBUF cast across
                            # DVE and ACT and store each half as it lands.
                            # The stores ride the sync ring (empty by now),
                            # so descriptor-gen overlaps the casts instead
                            # of queuing behind them on the ACT sequencer.
                            nc.vector.tensor_copy(
                                ybuf[:, :D // 2], psy[:, :D // 2])
                            nc.scalar.activation(
                                ybuf[:, D // 2:], psy[:, D // 2:],
                                mybir.ActivationFunctionType.Copy)
                            # halves issue from different sequencers so
                            # their DMA receipts overlap at exec end.
                            nc.sync.dma_start(yt[e, b][:, :D // 2],
                                              ybuf[:, :D // 2])
                            nc.scalar.dma_start(yt[e, b][:, D // 2:],
                                                ybuf[:, D // 2:])
                        else:
                            nc.vector.tensor_copy(ybuf[:], psy[:])
                            nc.scalar.dma_start(yt[e, b], ybuf[:])

    nc.compile()
    return nc


def _route(expert_idx):
    idx = np.asarray(expert_idx).astype(np.int64)
    order = np.argsort(idx, kind="stable")
    counts = np.bincount(idx, minlength=E)
    starts = np.zeros(E + 1, dtype=np.int64)
    starts[1:] = np.cumsum(counts)
    return order, starts, counts


def _pack_inputs(x, fc1_w, fc2_w, order, starts, C):
    x16 = x.astype(np.float16)
    in_maps = []
    for core in range(NCORES):
        w1h = np.empty((NG, 128, GPE * WE), np.float16)
        w2h = np.empty((EPC, 128, WE), np.float16)
        xh = np.zeros((128, EPC * DC * C), np.float16)
        for s in range(EPC):
            e = core * EPC + s
            g, sl = divmod(s, GPE)
            # W1^T = fc1_w[e].T : [D, H]; d = c*128 + p -> col c*H + h
            w1t = np.ascontiguousarray(fc1_w[e].T).reshape(DC, 128, H)
            w1h[g, :, sl * WE:(sl + 1) * WE] = (
                w1t.transpose(1, 0, 2).reshape(128, WE).astype(np.float16))
            # W2^T = fc2_w[e].T : [H, D]; h = ch*128 + p -> col ch*D + d
            w2t = np.ascontiguousarray(fc2_w[e].T).reshape(HC, 128, D)
            w2h[s] = w2t.transpose(1, 0, 2).reshape(128, WE).astype(np.float16)

            toks = order[starts[e]:starts[e + 1]]
            n = len(toks)
            if n:
                xte = np.ascontiguousarray(x16[toks].T).reshape(DC, 128, n)
                for c in range(DC):
                    base = (s * DC + c) * C
                    xh[:, base:base + n] = xte[c]
        in_maps.append({"w1t": w1h, "w2t": w2h, "xt": xh,
                        "idt": np.eye(128, dtype=np.float16)})
    return in_maps


def _unpack_outputs(results, order, starts, C, out_dtype):
    out = np.zeros((T, D), out_dtype)
    for core in range(NCORES):
        yh = np.asarray(results[core]["yt"]).astype(out_dtype)
        for s in range(EPC):
            e = core * EPC + s
            toks = order[starts[e]:starts[e + 1]]
            n = len(toks)
            if n:
                out[toks] = yh[s].reshape(C, D)[:n]
    return out


def kernel(x, expert_idx, fc1_w, fc2_w):
    from concourse.bass_utils import run_bass_kernel_spmd

    x = np.asarray(x, dtype=np.float32)
    fc1_w = np.asarray(fc1_w, dtype=np.float32)
    fc2_w = np.asarray(fc2_w, dtype=np.float32)

    order, starts, counts = _route(expert_idx)
    C = max(CB, int(-(-int(counts.max()) // CB) * CB))

    if C not in _prog_cache:
        _prog_cache[C] = _build_program(C)
    nc = _prog_cache[C]

    in_maps = _pack_inputs(x, fc1_w, fc2_w, order, starts, C)
    res = run_bass_kernel_spmd(nc, in_maps, list(range(NCORES)))
    return _unpack_outputs(res.results, order, starts, C, np.float32)
